# revision 6
# baseline (speedup 1.0000x reference)
"""Multi-head attention (B=8, N=1024, D=768, 12 heads x 64) on 8 TRN2
NeuronCores, batch-parallel (one batch element per core, no collectives).

v2: fully software-pipelined schedule.
  - v-projection kc-outer so compute starts as soon as the first x chunk
    lands (was: 11us dead time waiting for all input DMAs)
  - attention kc-loop software-pipelined: S^T(kc+1) issues before PV(kc),
    so the PE never waits on the scalar-engine exp latency chain
  - projection / out-projection matmuls distributed as "filler units"
    inside the attention loops (2 pairs ahead), keeping the PE busy during
    exp-paced stretches; ScalarE does nothing but the 96 exps
  - PV evictions + softmax-normalize DMAs ride the idle GpSimd engine's
    DMA port; reciprocal+broadcast per quad pipelined off the critical path
  - out-projection bias folded into the DVE eviction (tensor_scalar_add
    with a per-partition bias column); output written bf16
"""
import sys

sys.path.insert(0, "/opt/trn_rl_repo")

import numpy as np
import ml_dtypes

import concourse.bass as bass
import concourse.tile as tile_mod
from concourse import mybir
from concourse.bass_utils import run_bass_kernel_spmd
from concourse.vector_clock import ScopedClock

F32 = mybir.dt.float32
BF16 = mybir.dt.bfloat16

B, N, D = 8, 1024, 768
H, DH = 12, 64
HP = H // 2          # head pairs (two heads share a 128-partition tile)
KC = D // 128        # contraction chunks for the projections
RC = N // 128        # row chunks of the sequence
NK = N // 128        # key chunks
SCALE = DH ** -0.5


# --- walrus workaround: one sync-wait per instruction ---------------------
def _patched_drain_and_barrier(self, tick_clock, wait_clock):
    drain_inst = self.nc.sync.drain()
    wait_clock.add_sem_waits(
        drain_inst.ins, ScopedClock({None: tick_clock.global_clock})
    )
    si = drain_inst.ins.sync_info
    waits = list(si.on_wait or []) if si is not None else []
    if len(waits) > 1:
        drain_inst.ins.sync_info = mybir.SyncInfo(
            on_wait=waits[:1], on_update=list(si.on_update or [])
        )
        for w in waits[1:]:
            nop = self.nc.sync.nop(nofuse=True)
            nop.ins.sync_info = mybir.SyncInfo(on_wait=[w], on_update=[])
    self.nc.all_engine_barrier()
    assert self.sems is not None
    popped = self.nc._tile_sem_poison_stack.pop()
    assert popped is self._sem_poison
    self.nc.clear_and_free_semaphores(list(self.sems.allocated().values()))
    self.nc.all_engine_barrier()


tile_mod.TileContext._drain_and_barrier = _patched_drain_and_barrier


_split_counter = [0]


def split_sync_waits(nc, max_waits=1):
    """walrus rejects instructions carrying several sem waits; spill the
    excess onto engine-matched NOPs inserted directly before the offender."""
    for f in nc.m.functions:
        for bb in f.blocks:
            il = bb.instructions
            i = 0
            while i < len(il):
                inst = il[i]
                si = inst.sync_info
                waits = list(si.on_wait or []) if si is not None else []
                if len(waits) > max_waits:
                    inst.sync_info = mybir.SyncInfo(
                        on_wait=waits[:max_waits],
                        on_update=list(si.on_update or []),
                    )
                    rest = waits[max_waits:]
                    nops = []
                    for j in range(0, len(rest), max_waits):
                        _split_counter[0] += 1
                        nop = mybir.InstNoOp(
                            name=f"I-waitsplit-{_split_counter[0]}",
                            ins=[],
                            outs=[],
                            engine=inst.engine,
                        )
                        nop.sync_info = mybir.SyncInfo(
                            on_wait=rest[j : j + max_waits], on_update=[]
                        )
                        nops.append(nop)
                    for k, nop in enumerate(nops):
                        il.insert(i + k, nop)
                    i += len(nops)
                i += 1


def build_nc():
    nc = bass.Bass()
    xt_d = nc.dram_tensor("xt", [D, N], BF16, kind="ExternalInput")
    wq_d = nc.dram_tensor("wq", [D, 3 * D], BF16, kind="ExternalInput")
    wo_d = nc.dram_tensor("wo", [D, D], BF16, kind="ExternalInput")
    bo_d = nc.dram_tensor("bo2", [128 * KC], F32, kind="ExternalInput")
    cos_d = nc.dram_tensor("cos2", [128, N], BF16, kind="ExternalInput")
    sin_d = nc.dram_tensor("sin2", [128, N], BF16, kind="ExternalInput")
    perm_d = nc.dram_tensor("perm", [128, 128], BF16, kind="ExternalInput")
    out_d = nc.dram_tensor("out", [D, N], BF16, kind="ExternalOutput")
    import os as _os0

    _dbg = _os0.environ.get("K_DEBUG", "0") == "1"
    if _dbg:
        dbg_q = nc.dram_tensor("dbg_q", [128, KC, N], BF16, kind="ExternalOutput")
        dbg_k = nc.dram_tensor("dbg_k", [128, KC, N], BF16, kind="ExternalOutput")
        dbg_v = nc.dram_tensor(
            "dbg_v", [128, NK, H, DH + 1], BF16, kind="ExternalOutput"
        )
        dbg_au = nc.dram_tensor("dbg_au", [128, KC, N], F32, kind="ExternalOutput")
        dbg_sums = nc.dram_tensor("dbg_sums", [96, 128], F32, kind="ExternalOutput")
        dbg_attn = nc.dram_tensor("dbg_attn", [128, KC, N], BF16, kind="ExternalOutput")

    Exp = mybir.ActivationFunctionType.Exp
    Add = mybir.AluOpType.add

    with tile_mod.TileContext(nc) as tc:
        with (
            tc.tile_pool(name="singles", bufs=1) as singles,
            tc.tile_pool(name="apool", bufs=4) as apool,
            tc.tile_pool(name="bpool", bufs=2) as bpool,
            tc.tile_pool(name="dpool", bufs=1, space="DRAM") as dpool,
        ):
            # ---- input staging (sync engine; emission order = priority) --
            xt_sb = singles.tile([128, KC, N], BF16)
            wv_sb = singles.tile([128, KC, D], BF16)
            for kc in range(KC):
                nc.sync.dma_start(
                    out=xt_sb[:, kc, :], in_=xt_d[kc * 128 : (kc + 1) * 128, :]
                )
                nc.sync.dma_start(
                    out=wv_sb[:, kc, :],
                    in_=wq_d[kc * 128 : (kc + 1) * 128, 2 * D : 3 * D],
                )
            # batched q/k weight tiles: one 3D DMA per 128-col tile
            wqt = []
            for oc in range(2 * KC):
                col0 = oc * 128 if oc < KC else D + (oc - KC) * 128
                wt = singles.tile([128, KC, 128], BF16, name=f"wqt{oc}")
                nc.sync.dma_start(
                    out=wt[:],
                    in_=wq_d[:, col0 : col0 + 128].rearrange(
                        "(kc p) c -> p kc c", p=128
                    ),
                )
                wqt.append(wt)
            cos_sb = singles.tile([128, N], BF16)
            nc.sync.dma_start(out=cos_sb[:], in_=cos_d[:])
            sin_sb = singles.tile([128, N], BF16)
            nc.sync.dma_start(out=sin_sb[:], in_=sin_d[:])
            perm_sb = singles.tile([128, 128], BF16)
            nc.sync.dma_start(out=perm_sb[:], in_=perm_d[:])
            bo2_sb = singles.tile([128, KC], F32)
            nc.sync.dma_start(
                out=bo2_sb[:], in_=bo_d[:].rearrange("(p c) -> p c", p=128)
            )
            wo_sb = singles.tile([128, KC, D], BF16)
            for c in range(KC):
                nc.sync.dma_start(
                    out=wo_sb[:, c, :], in_=wo_d[c * 128 : (c + 1) * 128, :]
                )

            v_sb = singles.tile([128, NK, H, DH + 1], BF16)
            nc.gpsimd.memset(v_sb[:, :, :, DH : DH + 1], 1.0)

            q_sb = singles.tile([128, KC, N], BF16)
            k_sb = singles.tile([128, KC, N], BF16)
            attnU_sb = singles.tile([128, KC, N], F32)
            attn_sb = singles.tile([128, KC, N], BF16)
            # per query-half sums tiles; quad q4 occupies the 32-aligned row
            # block [32*q4, 32*q4+32) as [h_rel*8 + j, c] -> head 4*q4+h_rel,
            # query j*64+c (reciprocal requires 32-aligned base partitions)
            sums_q = [singles.tile([96, 64], F32, name=f"sums{qc}") for qc in range(2)]
            recip_q = [singles.tile([96, 64], F32, name=f"recip{qc}") for qc in range(2)]
            partial_sb = singles.tile([128, KC, 512], F32)
            recip_d = dpool.tile([2 * H * 512], F32)

            # ---- v projection, kc-outer: starts after first x chunk -----
            with tc.tile_pool(name="ps_v", bufs=1, space="PSUM") as ps_v:
                for p in range(2):
                    vps = [
                        ps_v.tile([128, D], F32, tag=f"v{i}", name=f"vp{p}_{i}")
                        for i in range(4)
                    ]
                    for kc in range(KC):
                        for i in range(4):
                            rc = p * 4 + i
                            for c0, w in ((0, 512), (512, 256)):
                                nc.tensor.matmul(
                                    vps[i][:, c0 : c0 + w],
                                    xt_sb[:, kc, rc * 128 : (rc + 1) * 128],
                                    wv_sb[:, kc, c0 : c0 + w],
                                    start=(kc == 0),
                                    stop=(kc == KC - 1),
                                )
                    for i in range(4):
                        nc.vector.tensor_copy(
                            v_sb[:, p * 4 + i, :, 0:DH],
                            vps[i][:].rearrange("p (h d) -> p h d", h=H),
                        )

            # ---- filler-unit machinery ----------------------------------
            from collections import deque

            filler_q = deque()

            def popf(n):
                for _ in range(n):
                    if not filler_q:
                        return
                    filler_q.popleft()()

            def run_units(units):
                for u in units:
                    u()

            # ---- q^T / k^T projection + RoPE, per 512-col half ----------
            def proj_units(ps_qk, oc):
                col0_unused = oc  # captured below
                dst = q_sb if oc < KC else k_sb
                state = {}

                def mk_qkp(qc2, kcs):
                    def f():
                        key = f"qkp{oc}_{qc2}"
                        if key not in state:
                            state[key] = ps_qk.tile(
                                [128, 512], F32, tag="qk", name=key
                            )
                        for kc in kcs:
                            nc.tensor.matmul(
                                state[key][:],
                                wqt[oc][:, kc, :],
                                xt_sb[:, kc, qc2 * 512 : (qc2 + 1) * 512],
                                start=(kc == 0),
                                stop=(kc == KC - 1),
                            )
                    return f

                def mk_rot(qc2):
                    def f():
                        q0 = apool.tile(
                            [128, 512], BF16, tag="q0", name=f"q0_{oc}_{qc2}"
                        )
                        nc.vector.tensor_copy(q0[:], state[f"qkp{oc}_{qc2}"][:])
                        state[f"q0_{qc2}"] = q0
                        rotp = ps_qk.tile(
                            [128, 512], F32, tag="qk", name=f"rotp{oc}_{qc2}"
                        )
                        nc.tensor.matmul(
                            rotp[:], perm_sb[:], q0[:], start=True, stop=True
                        )
                        state[f"rotp_{qc2}"] = rotp
                    return f

                def mk_comb(qc2):
                    def f():
                        cols = slice(qc2 * 512, (qc2 + 1) * 512)
                        q0 = state[f"q0_{qc2}"]
                        rotp = state[f"rotp_{qc2}"]
                        t1 = apool.tile(
                            [128, 512], BF16, tag="t1", name=f"t1_{oc}_{qc2}"
                        )
                        nc.vector.tensor_mul(t1[:], rotp[:], sin_sb[:, cols])
                        t2 = apool.tile(
                            [128, 512], BF16, tag="t2", name=f"t2_{oc}_{qc2}"
                        )
                        nc.vector.tensor_mul(t2[:], q0[:], cos_sb[:, cols])
                        nc.vector.tensor_add(dst[:, oc % KC, cols], t1[:], t2[:])
                    return f

                units = []
                for qc2 in range(2):
                    units.append(mk_qkp(qc2, (0, 1, 2)))
                    units.append(mk_qkp(qc2, (3, 4, 5)))
                    units.append(mk_rot(qc2))
                    units.append(mk_comb(qc2))
                return units

            # ---- out-projection units -----------------------------------
            def outproj_units(ps_fin, oc, qc, c_lo, c_hi, mode):
                """mode: 'full' -> evict with bias to out; 'partial' ->
                evict with bias to partial_sb; 'combine' -> add partial."""
                state = {}

                def mk_mm(cs):
                    def f():
                        key = "fin"
                        if key not in state:
                            state[key] = ps_fin.tile(
                                [128, 512], F32, tag="fin",
                                name=f"fin{qc}_{oc}_{c_lo}",
                            )
                        for c in cs:
                            nc.tensor.matmul(
                                state[key][:],
                                wo_sb[:, c, oc * 128 : (oc + 1) * 128],
                                attn_sb[:, c, qc * 512 : (qc + 1) * 512],
                                start=(c == c_lo),
                                stop=(c == c_hi - 1),
                            )
                    return f

                def mk_evict():
                    def f():
                        fin = state["fin"]
                        if mode == "partial":
                            nc.vector.tensor_scalar_add(
                                partial_sb[:, oc, :], fin[:], bo2_sb[:, oc : oc + 1]
                            )
                            return
                        ob = bpool.tile(
                            [128, 512], BF16, tag="ob", bufs=3,
                            name=f"ob{qc}_{oc}_{c_lo}",
                        )
                        if mode == "combine":
                            nc.vector.tensor_add(
                                ob[:], fin[:], partial_sb[:, oc, :]
                            )
                        else:
                            nc.vector.tensor_scalar_add(
                                ob[:], fin[:], bo2_sb[:, oc : oc + 1]
                            )
                        nc.sync.dma_start(
                            out=out_d[
                                oc * 128 : (oc + 1) * 128,
                                qc * 512 : (qc + 1) * 512,
                            ],
                            in_=ob[:],
                        )
                    return f

                cs = list(range(c_lo, c_hi))
                units = []
                for j in range(0, len(cs), 2):
                    units.append(mk_mm(cs[j : j + 2]))
                units.append(mk_evict())
                return units

            # ---- attention: software-pipelined head pairs ---------------
            def attn_pair(ps_att, qc, hp):
                pvs = [
                    ps_att.tile(
                        [65, 512], F32, tag=f"pv{a}", bufs=1,
                        name=f"pv{a}_{qc}_{hp}",
                    )
                    for a in range(2)
                ]
                sts = {}

                def emit_S(kc):
                    st = ps_att.tile(
                        [128, N], F32, tag="st", bufs=2, name=f"st_{qc}_{hp}_{kc}"
                    )
                    sts[kc] = st
                    for a in range(2):
                        po = 64 * a
                        nc.tensor.matmul(
                            st[:, a * 512 : (a + 1) * 512],
                            k_sb[po : po + 64, hp, kc * 128 : (kc + 1) * 128],
                            q_sb[po : po + 64, hp, qc * 512 : (qc + 1) * 512],
                            start=True,
                            stop=True,
                        )

                emit_S(0)
                for kc in range(NK):
                    if kc + 1 < NK:
                        emit_S(kc + 1)
                    e = apool.tile(
                        [128, N], BF16, tag="e", name=f"e_{qc}_{hp}_{kc}"
                    )
                    nc.scalar.activation(
                        out=e[:], in_=sts[kc][:], func=Exp, scale=SCALE
                    )
                    for a in range(2):
                        nc.tensor.matmul(
                            pvs[a][:],
                            v_sb[:, kc, 2 * hp + a, :],
                            e[:, a * 512 : (a + 1) * 512],
                            start=(kc == 0),
                            stop=(kc == NK - 1),
                        )
                    popf(2)
                # evict: DVE copy to SBUF, then gpsimd-issued DMAs unpack
                for a in range(2):
                    h = 2 * hp + a
                    po = 64 * a
                    pvt = bpool.tile(
                        [65, 512], F32, tag="pvt", bufs=3, name=f"pvt{qc}_{h}"
                    )
                    nc.vector.tensor_copy(pvt[:], pvs[a][:])
                    nc.gpsimd.dma_start(
                        out=attnU_sb[po : po + 64, hp, qc * 512 : (qc + 1) * 512],
                        in_=pvt[0:64, :],
                    )
                    r0 = (h // 4) * 32 + (h % 4) * 8
                    nc.gpsimd.dma_start(
                        out=sums_q[qc][r0 : r0 + 8, :],
                        in_=pvt[64:65, :],
                    )

            def normalize_quad(qc, q4):
                r0 = 32 * q4
                nc.vector.reciprocal(
                    recip_q[qc][r0 : r0 + 32, :], sums_q[qc][r0 : r0 + 32, :]
                )
                nc.gpsimd.dma_start(
                    out=recip_d[
                        (qc * H + 4 * q4) * 512 : (qc * H + 4 * q4 + 4) * 512
                    ],
                    in_=recip_q[qc][r0 : r0 + 32, :],
                )
                for hp2 in (2 * q4, 2 * q4 + 1):
                    rb = bpool.tile(
                        [128, 512], F32, tag="rb", bufs=2, name=f"rb{qc}_{hp2}"
                    )
                    src = bass.AP(
                        tensor=recip_d[:].tensor,
                        offset=recip_d[:].offset + (qc * H + 2 * hp2) * 512,
                        ap=[[512, 2], [0, 64], [1, 512]],
                    )
                    nc.gpsimd.dma_start(out=rb[:], in_=src)
                    cols = slice(qc * 512, (qc + 1) * 512)
                    nc.vector.tensor_mul(
                        attn_sb[:, hp2, cols], attnU_sb[:, hp2, cols], rb[:]
                    )

            # ---- main schedule ------------------------------------------
            with tc.tile_pool(name="ps_att", bufs=1, space="PSUM") as ps_att:
                with tc.tile_pool(name="ps_qk", bufs=2, space="PSUM") as ps_qk:
                    # upfront projections: pairs 0 and 1 ready before B
                    for oc in (0, KC, 1, KC + 1):
                        run_units(proj_units(ps_qk, oc))
                    # phase B: attention qc=0, proj fillers 2 pairs ahead
                    for hp in range(HP):
                        if hp + 2 < HP:
                            filler_q.extend(proj_units(ps_qk, hp + 2))
                            filler_q.extend(proj_units(ps_qk, KC + hp + 2))
                        attn_pair(ps_att, 0, hp)
                        if hp % 2 == 1:
                            normalize_quad(0, hp // 2)
                        popf(6)
                    while filler_q:
                        popf(8)
                with tc.tile_pool(name="ps_fin", bufs=2, space="PSUM") as ps_fin:
                    # phase C: attention qc=1; fillers = out-proj qc0, then
                    # the c:0-2 partial sweep of out-proj qc1
                    for hp in range(HP):
                        if hp == 1:
                            for oc in range(KC):
                                filler_q.extend(
                                    outproj_units(ps_fin, oc, 0, 0, KC, "full")
                                )
                        if hp == 4:
                            for oc in range(KC):
                                filler_q.extend(
                                    outproj_units(ps_fin, oc, 1, 0, 3, "partial")
                                )
                        attn_pair(ps_att, 1, hp)
                        if hp % 2 == 1:
                            normalize_quad(1, hp // 2)
                        popf(6)
                    while filler_q:
                        popf(8)
                    # tail: out-proj qc1 c:3-5 + combine with partials
                    for oc in range(KC):
                        run_units(outproj_units(ps_fin, oc, 1, 3, KC, "combine"))

            if _dbg:
                nc.sync.dma_start(out=dbg_q[:], in_=q_sb[:])
                nc.sync.dma_start(out=dbg_k[:], in_=k_sb[:])
                nc.sync.dma_start(out=dbg_v[:], in_=v_sb[:])
                nc.sync.dma_start(out=dbg_au[:], in_=attnU_sb[:])
                nc.sync.dma_start(out=dbg_sums[:, 0:64], in_=sums_q[0][:])
                nc.sync.dma_start(out=dbg_sums[:, 64:128], in_=sums_q[1][:])
                nc.sync.dma_start(out=dbg_attn[:], in_=attn_sb[:])

    split_sync_waits(nc, max_waits=1)
    return nc


def _host_prep(x, w_qkv, w_out, b_out):
    bf = ml_dtypes.bfloat16
    inv_freq = 1.0 / (10000.0 ** (np.arange(0, DH, 2, dtype=np.float32) / DH))
    t = np.arange(N, dtype=np.float32)
    freqs = np.outer(t, inv_freq)
    emb = np.concatenate([freqs, freqs], axis=1)        # [N, DH]
    cos2 = np.tile(np.cos(emb).T.astype(np.float32), (2, 1)).astype(bf)
    sin2 = np.tile(np.sin(emb).T.astype(np.float32), (2, 1)).astype(bf)

    perm = np.zeros((128, 128), np.float32)
    for blk in range(2):
        o = blk * 64
        for m in range(32):
            perm[o + m + 32, o + m] = -1.0
        for m in range(32, 64):
            perm[o + m - 32, o + m] = 1.0
    perm = perm.astype(bf)

    # bias in [128, KC] partition-major layout: bo2[p, c] = b_out[c*128+p]
    bo2 = np.ascontiguousarray(
        b_out.reshape(KC, 128).T.astype(np.float32)
    ).reshape(-1)

    xt = np.ascontiguousarray(x.transpose(0, 2, 1)).astype(bf)
    shared = {
        "wq": np.ascontiguousarray(w_qkv).astype(bf),
        "wo": np.ascontiguousarray(w_out).astype(bf),
        "bo2": bo2,
        "cos2": np.ascontiguousarray(cos2),
        "sin2": np.ascontiguousarray(sin2),
        "perm": np.ascontiguousarray(perm),
    }
    return [dict(shared, xt=np.ascontiguousarray(xt[i])) for i in range(B)]


_NC_CACHE = {}
LAST_EXEC_NS = [None]


def _run(in_maps, trace=False):
    if "nc" not in _NC_CACHE:
        _NC_CACHE["nc"] = build_nc()
    res = run_bass_kernel_spmd(
        _NC_CACHE["nc"], in_maps, list(range(B)), trace=trace
    )
    LAST_EXEC_NS[0] = res.exec_time_ns
    out_t = np.stack(
        [np.asarray(res.results[i]["out"]).astype(np.float32) for i in range(B)]
    )
    return np.ascontiguousarray(out_t.transpose(0, 2, 1))


def kernel(x, w_qkv, w_out, b_out, _trace=False):
    in_maps = _host_prep(
        np.asarray(x, dtype=np.float32),
        np.asarray(w_qkv, dtype=np.float32),
        np.asarray(w_out, dtype=np.float32),
        np.asarray(b_out, dtype=np.float32),
    )
    return _run(in_maps, trace=_trace)
